# revision 14
# baseline (speedup 1.0000x reference)
"""Differential attention (dense_transformer) Trainium2 kernel.

Full-input contract: kernel(**inputs) takes the unsharded inputs of
reference.setup_inputs() and returns the full (1, S, D) float32 output.

Sharding: 16 heads across 8 cores (2 heads/core, tensor-parallel on the
q/k/v projection rows and wo columns). Each core computes a full (S, D)
partial of the output projection; the host sums partials and adds the
residual.
"""

import sys

for _p in ("/opt/trn_rl_repo", "/root/.axon_site/_ro/trn_rl_repo"):
    if _p not in sys.path:
        sys.path.insert(0, _p)

import math

import numpy as np

import concourse.bass as bass
import concourse.mybir as mybir
import concourse.tile as tile
from concourse import bacc
from concourse.bass import ts
from concourse.bass_utils import run_bass_kernel_spmd
from concourse.masks import make_identity, make_upper_triangular

F32 = mybir.dt.float32
F16 = mybir.dt.float16

# Problem constants
B, S, D = 1, 2048, 2048
H, C, HD = 16, 2, 128
DM = HD * C  # 256 per-head q/k dim
N_CORES = 8
HPC = H // N_CORES  # heads per core = 2
NHC = HPC * C  # head-comp blocks per core = 4
EPS = 1e-9
CONST = 10000.0
SQ = 512  # S_q super-tile width


def build_kernel(s=S):
    """Build the per-core Bass kernel (SPMD; per-core data differs)."""
    ns = s // 128  # S chunks of 128
    nj = s // SQ  # S_q super tiles
    kd = D // 128  # contraction chunks over D

    nc = bacc.Bacc("TRN2", target_bir_lowering=False, debug=False,
                   num_devices=N_CORES)

    x_d = nc.dram_tensor("x", [s, D], F32, kind="ExternalInput")
    wqt_d = nc.dram_tensor("wqt", [D, NHC * 128], F16, kind="ExternalInput")
    wkt_d = nc.dram_tensor("wkt", [D, NHC * 128], F16, kind="ExternalInput")
    wvt_d = nc.dram_tensor("wvt", [D, HPC * HD], F16, kind="ExternalInput")
    wot_d = nc.dram_tensor("wot", [HPC * HD, D], F16, kind="ExternalInput")
    cost_d = nc.dram_tensor("cost", [128, s], F16, kind="ExternalInput")
    sint_d = nc.dram_tensor("sint", [128, s], F16, kind="ExternalInput")
    lam_d = nc.dram_tensor("lam", [1, HPC], F32, kind="ExternalInput")
    out_d = nc.dram_tensor("out", [s, D], F32, kind="ExternalOutput")

    inv_sqrt_hd = 1.0 / math.sqrt(HD)

    with tile.TileContext(nc) as tc:
        with (
            tc.tile_pool(name="const", bufs=1) as cp,
            tc.tile_pool(name="qk", bufs=1) as qkp,
            tc.tile_pool(name="vat", bufs=1) as vap,
        ):
            # ---- small persistent constants ----
            lam = cp.tile([128, HPC], F32, tag="lam")
            _lap = lam_d[:, :]
            nc.sync.dma_start(
                out=lam,
                in_=bass.AP(tensor=_lap.tensor, offset=_lap.offset,
                            ap=[[0, 128]] + list(_lap.ap)[1:]),
            )
            m0 = cp.tile([128, 128], F16, tag="m0")
            make_upper_triangular(nc, m0, val=1.0, diag=True)
            ident = cp.tile([128, 128], F16, tag="ident")
            make_identity(nc, ident)
            epsc = cp.tile([128, 1], F32, tag="epsc")
            nc.vector.memset(epsc, EPS)

            # persistent activations
            qT = qkp.tile([128, NHC, s], F16, tag="qT")
            kT = qkp.tile([128, NHC, s], F16, tag="kT")
            vaug = vap.tile([128, HPC, ns, 132], F16, tag="vaug")

            with (
                tc.tile_pool(name="wqkv", bufs=1) as wp,
                tc.tile_pool(name="ht", bufs=1) as htp,
            ):
                wqt = wp.tile([128, kd, NHC * 128], F16, tag="wqt")
                nc.sync.dma_start(out=wqt,
                                  in_=wqt_d.rearrange("(k p) m -> p k m", p=128))
                wkt = wp.tile([128, kd, NHC * 128], F16, tag="wkt")
                nc.sync.dma_start(out=wkt,
                                  in_=wkt_d.rearrange("(k p) m -> p k m", p=128))
                wvt = wp.tile([128, kd, HPC * HD], F16, tag="wvt")
                nc.sync.dma_start(out=wvt,
                                  in_=wvt_d.rearrange("(k p) m -> p k m", p=128))
                ht = htp.tile([128, kd, s], F16, tag="ht")

                # ---- phase 1: x -> rmsnorm -> h16 -> hT ----
                with (
                    tc.tile_pool(name="ph1", bufs=2) as p1,
                    tc.tile_pool(name="stats", bufs=4) as stp,
                ):
                    for i in range(ns):
                        xt = p1.tile([128, D], F32, tag="xt")
                        nc.sync.dma_start(out=xt, in_=x_d[ts(i, 128), :])
                        acc = stp.tile([128, 1], F32, tag="acc")
                        h16 = p1.tile([128, D], F16, tag="h16")
                        nc.scalar.activation(
                            out=h16, in_=xt,
                            func=mybir.ActivationFunctionType.Square,
                            accum_out=acc)
                        rcol = stp.tile([128, 1], F32, tag="rcol")
                        nc.scalar.activation(out=rcol, in_=acc,
                                             func=mybir.ActivationFunctionType.Sqrt,
                                             scale=1.0 / D, bias=epsc)
                        nc.vector.reciprocal(out=rcol, in_=rcol)
                        nc.scalar.activation(out=h16, in_=xt,
                                             func=mybir.ActivationFunctionType.Copy,
                                             scale=rcol)
                        nc.sync.dma_start_transpose(out=ht[:, :, ts(i, 128)],
                                                    in_=h16)

                # ---- phase 2: projections + RoPE + repack ----
                # q/k are projected into a "split" row layout
                # [R0, R1, I0, I1] (R = rope-real rows, I = rope-imag rows;
                # j in {0,1} indexes the two 128-row groups of real parts).
                # RoPE then runs full-lane with partition-aligned operands,
                # and SBUF->SBUF DMAs repack into per-head-comp [xr;xi]
                # tiles (qT/kT) for K=128 attention matmuls.
                mul = mybir.AluOpType.mult
                with tc.tile_pool(name="pps", bufs=4, space="PSUM") as pps, \
                     tc.tile_pool(name="split", bufs=1) as spp, \
                     tc.tile_pool(name="rope", bufs=2) as rp, \
                     tc.tile_pool(name="ropec", bufs=1) as rcp:
                    cost = rcp.tile([128, s], F16, tag="cost")
                    nc.sync.dma_start(out=cost, in_=cost_d[:, :])
                    sint = rcp.tile([128, s], F16, tag="sint")
                    nc.sync.dma_start(out=sint, in_=sint_d[:, :])
                    for w_sb, t_sb in ((wqt, qT), (wkt, kT)):
                        qs = spp.tile([128, 4, s], F16, tag="qs")
                        for mb in range(4):
                            for j in range(nj):
                                ps = pps.tile([128, SQ], F32, tag="ps")
                                for k in range(kd):
                                    nc.tensor.matmul(
                                        ps, w_sb[:, k, ts(mb, 128)],
                                        ht[:, k, ts(j, SQ)],
                                        start=(k == 0), stop=(k == kd - 1))
                                nc.vector.tensor_copy(out=qs[:, mb, ts(j, SQ)],
                                                      in_=ps)
                        # RoPE in place on (R_j, I_j) pairs, full 128 lanes
                        for j2 in range(2):
                            xr = qs[:, j2, :]
                            xi = qs[:, j2 + 2, :]
                            t2 = rp.tile([128, s], F16, tag="t2")
                            t3 = rp.tile([128, s], F16, tag="t3")
                            nc.vector.tensor_tensor(out=t2, in0=xi, in1=sint,
                                                    op=mul)
                            nc.vector.tensor_tensor(out=t3, in0=xr, in1=sint,
                                                    op=mul)
                            nc.vector.tensor_tensor(out=xr, in0=xr, in1=cost,
                                                    op=mul)
                            nc.vector.tensor_tensor(out=xr, in0=xr, in1=t2,
                                                    op=mybir.AluOpType.subtract)
                            nc.vector.tensor_tensor(out=xi, in0=xi, in1=cost,
                                                    op=mul)
                            nc.vector.tensor_tensor(out=xi, in0=xi, in1=t3,
                                                    op=mybir.AluOpType.add)
                        # repack: hc tile = [xr(64) ; xi(64)]
                        for hc in range(NHC):
                            j2, half = hc // 2, hc % 2
                            nc.sync.dma_start(
                                out=t_sb[0:64, hc, :],
                                in_=qs[ts(half, 64), j2, :])
                            nc.sync.dma_start(
                                out=t_sb[64:128, hc, :],
                                in_=qs[ts(half, 64), j2 + 2, :])
                    for i in range(ns):
                        ps = pps.tile([128, HPC * HD], F32, tag="vps")
                        for k in range(kd):
                            nc.tensor.matmul(ps, ht[:, k, ts(i, 128)], wvt[:, k, :],
                                             start=(k == 0), stop=(k == kd - 1))
                        for h in range(HPC):
                            nc.vector.tensor_copy(out=vaug[:, h, i, 0:128],
                                                  in_=ps[:, ts(h, 128)])
                    for h in range(HPC):
                        nc.vector.memset(vaug[:, h, :, 128:129], 1.0)

            # ---- phase 3: attention ----
            attT = qkp.tile([128, HPC, s], F16, tag="attT")
            with (
                tc.tile_pool(name="ep", bufs=17) as ep,
                tc.tile_pool(name="qkps", bufs=2, space="PSUM") as qkps,
                tc.tile_pool(name="avps", bufs=4, space="PSUM") as avps,
                tc.tile_pool(name="tpps", bufs=2, space="PSUM") as tpps,
                tc.tile_pool(name="comb", bufs=2) as cbp,
                tc.tile_pool(name="attc", bufs=2) as atcp,
                tc.tile_pool(name="small", bufs=8) as smp,
            ):
                mul = mybir.AluOpType.mult
                add = mybir.AluOpType.add
                for head in range(HPC):
                    for j in range(nj):
                        avsb = []
                        for c2 in range(C):
                            hc = C * head + c2
                            nblk = 4 * j + 4
                            es = []
                            for i in range(nblk):
                                r = i - 4 * j
                                c0 = 128 * max(r, 0)
                                eps_ = qkps.tile([128, SQ], F32, tag="eps")
                                nc.tensor.matmul(
                                    eps_[:, c0:SQ], kT[:, hc, ts(i, 128)],
                                    qT[:, hc, SQ * j + c0:SQ * j + SQ],
                                    start=True, stop=True)
                                et = ep.tile([128, SQ], F16, tag="et")
                                nc.scalar.activation(
                                    out=et[:, c0:SQ], in_=eps_[:, c0:SQ],
                                    func=mybir.ActivationFunctionType.Exp,
                                    scale=inv_sqrt_hd)
                                if r >= 0:
                                    nc.vector.tensor_tensor(
                                        out=et[:, c0:c0 + 128],
                                        in0=et[:, c0:c0 + 128], in1=m0, op=mul)
                                es.append(et)
                            attc = atcp.tile([128, 4, 132], F32, tag=f"attc{c2}")
                            avsb.append(attc)
                            for m in range(4):
                                avm = avps.tile([128, 129], F32, tag="avm")
                                for i in range(4 * j + m + 1):
                                    nc.tensor.matmul(
                                        avm, es[i][:, ts(m, 128)],
                                        vaug[:, head, i, 0:129],
                                        start=(i == 0), stop=(i == 4 * j + m))
                                nc.vector.tensor_copy(out=attc[:, m, 0:129],
                                                      in_=avm)
                        # combine the two components
                        A0, D0 = avsb[0][:, :, 0:128], avsb[0][:, :, 128:129]
                        A1, D1 = avsb[1][:, :, 0:128], avsb[1][:, :, 128:129]
                        prod = smp.tile([128, 4, 1], F32, tag="prod")
                        nc.vector.tensor_tensor(out=prod, in0=D0, in1=D1, op=mul)
                        epst = smp.tile([128, 4, 1], F32, tag="epst")
                        nc.vector.tensor_tensor(out=epst, in0=prod, in1=prod, op=mul)
                        nc.vector.tensor_scalar_mul(out=epst, in0=epst, scalar1=EPS)
                        d0l = smp.tile([128, 4, 1], F32, tag="d0l")
                        nc.vector.tensor_scalar_mul(
                            out=d0l, in0=D0, scalar1=lam[:, head:head + 1])
                        comb = cbp.tile([128, 4, 128], F32, tag="comb")
                        nc.vector.tensor_tensor(
                            out=comb, in0=A0, in1=D1.to_broadcast((128, 4, 128)),
                            op=mul)
                        tt = cbp.tile([128, 4, 128], F32, tag="tt")
                        nc.vector.tensor_tensor(
                            out=tt, in0=A1, in1=d0l.to_broadcast((128, 4, 128)),
                            op=mul)
                        nc.vector.tensor_tensor(out=comb, in0=comb, in1=tt, op=add)
                        nc.vector.tensor_tensor(out=tt, in0=comb, in1=comb, op=mul)
                        ssum = smp.tile([128, 4, 1], F32, tag="ssum")
                        nc.vector.reduce_sum(out=ssum, in_=tt,
                                             axis=mybir.AxisListType.X)
                        nc.vector.tensor_scalar_mul(out=ssum, in0=ssum,
                                                    scalar1=1.0 / HD)
                        nc.vector.tensor_tensor(out=ssum, in0=ssum, in1=epst, op=add)
                        rf = smp.tile([128, 4, 1], F32, tag="rf")
                        nc.scalar.activation(out=rf, in_=ssum,
                                             func=mybir.ActivationFunctionType.Sqrt)
                        nc.vector.reciprocal(out=rf, in_=rf)
                        a16 = cbp.tile([128, 4, 128], F16, tag="a16")
                        nc.vector.tensor_tensor(
                            out=a16, in0=comb, in1=rf.to_broadcast((128, 4, 128)),
                            op=mul)
                        for mm in range(4):
                            tp = tpps.tile([128, 128], F16, tag="tp")
                            nc.tensor.transpose(tp, a16[:, mm, :], ident)
                            nc.vector.tensor_copy(
                                out=attT[:, head,
                                         SQ * j + 128 * mm:SQ * j + 128 * mm + 128],
                                in_=tp)

            # ---- phase 4: output projection ----
            with (
                tc.tile_pool(name="wo", bufs=1) as wop,
                tc.tile_pool(name="ops", bufs=4, space="PSUM") as opsp,
                tc.tile_pool(name="ost", bufs=3) as ostp,
            ):
                wot = wop.tile([128, HPC, D], F16, tag="wot")
                nc.sync.dma_start(out=wot,
                                  in_=wot_d.rearrange("(h p) n -> p h n", p=128))
                for sm in range(ns):
                    for dn in range(D // SQ):
                        ps = opsp.tile([128, SQ], F32, tag="ops")
                        for h in range(HPC):
                            nc.tensor.matmul(ps, attT[:, h, ts(sm, 128)],
                                             wot[:, h, ts(dn, SQ)],
                                             start=(h == 0), stop=(h == HPC - 1))
                        ost = ostp.tile([128, SQ], F32, tag="ost")
                        nc.vector.tensor_copy(out=ost, in_=ps)
                        nc.sync.dma_start(out=out_d[ts(sm, 128), ts(dn, SQ)],
                                          in_=ost)

    nc.compile()
    return nc


def _perm_core():
    """Row permutation of one core's HPC*DM q/k rows into the split layout
    [R0..R_{HPC-1}, I0..I_{HPC-1}]: R_h = rope-real (even) rows of head h for
    both components, I_h = rope-imag (odd) rows. Within each 128-row block,
    rows follow theta-pair order 0..127."""
    evens = [h * DM + 128 * c + 2 * t
             for h in range(HPC) for c in range(C) for t in range(64)]
    odds = [h * DM + 128 * c + 2 * t + 1
            for h in range(HPC) for c in range(C) for t in range(64)]
    return np.array(evens + odds)


def prep_inputs(x, pre_norm_w, wq, wk, wv, wo, head_norm_w, q1, q2, k1, k2,
                lam_init, s=S):
    """Host-side prep: fold norms/lambdas into weights, permute q/k rows,
    transpose, cast fp16, build rope tables; returns per-core input maps."""
    x2 = np.asarray(x, np.float32).reshape(s, D)
    pw = np.asarray(pre_norm_w, np.float32)
    hw = np.asarray(head_norm_w, np.float32)
    li = np.asarray(lam_init, np.float64)

    wq_e = (np.asarray(wq, np.float64) * pw[None, :])
    wk_e = (np.asarray(wk, np.float64) * pw[None, :])
    wv_e = (np.asarray(wv, np.float64) * pw[None, :])
    # wo: out = att_normed * (1-lam) @ wo.T ; head_norm_w folds per att dim
    colscale = np.concatenate(
        [hw.astype(np.float64) * (1.0 - li[h]) for h in range(H)])
    wo_e = np.asarray(wo, np.float64) * colscale[None, :]

    base = (np.exp(np.sum(np.asarray(q1, np.float64) * np.asarray(k1, np.float64),
                          axis=-2))
            - np.exp(np.sum(np.asarray(q2, np.float64) * np.asarray(k2, np.float64),
                            axis=-2)))  # (H, 1)
    scale_h = -(H * base[:, 0] + li.sum())  # (H,)

    theta = 1.0 / (CONST ** (np.arange(0, DM, 2, dtype=np.float64) / DM))
    ang = np.arange(s, dtype=np.float64)[:, None] * theta[None, :]  # (s, 128)
    cost = np.cos(ang).T.astype(np.float16)  # (128, s)
    sint = np.sin(ang).T.astype(np.float16)

    ph = _perm_core()
    in_maps = []
    for core in range(N_CORES):
        heads = range(core * HPC, (core + 1) * HPC)
        rows = core * HPC * DM + ph
        wqt = np.ascontiguousarray(wq_e[rows].T).astype(np.float16)
        wkt = np.ascontiguousarray(wk_e[rows].T).astype(np.float16)
        vrows = np.concatenate(
            [np.arange(h * HD, (h + 1) * HD) for h in heads])
        wvt = np.ascontiguousarray(wv_e[vrows].T).astype(np.float16)
        wot = np.ascontiguousarray(wo_e[:, vrows].T).astype(np.float16)
        lamc = scale_h[list(heads)].astype(np.float32).reshape(1, HPC)
        in_maps.append({
            "x": x2, "wqt": wqt, "wkt": wkt, "wvt": wvt, "wot": wot,
            "cost": cost, "sint": sint, "lam": lamc,
        })
    return in_maps


_NC_CACHE = {}


def kernel(x, pre_norm_w, wq, wk, wv, wo, head_norm_w, q1, q2, k1, k2,
           lam_init):
    s = x.shape[1]
    if s not in _NC_CACHE:
        _NC_CACHE[s] = build_kernel(s)
    nc = _NC_CACHE[s]
    in_maps = prep_inputs(x, pre_norm_w, wq, wk, wv, wo, head_norm_w,
                          q1, q2, k1, k2, lam_init, s=s)
    res = run_bass_kernel_spmd(nc, in_maps, list(range(N_CORES)))
    acc = np.zeros((s, D), np.float64)
    for c in range(N_CORES):
        acc += res.results[c]["out"].astype(np.float64)
    out = acc.astype(np.float32) + np.asarray(x, np.float32).reshape(s, D)
    return out.reshape(1, s, D)
